# revision 9
# baseline (speedup 1.0000x reference)
"""GCN (2x GCNConv + BN/ReLU) -> global_mean_pool -> MLP head, on 8 TRN2 NeuronCores.

Strategy (graph/data parallel, per sharding hint):
- Nodes partitioned contiguously across 8 cores (6250 each, padded to 6272 = 49*128).
- Edges (incl. self-loops) sharded by destination node; per-core lists sorted by
  destination, bucketed into 49 blocks of 128 destinations, padded to a uniform
  T tiles of 128 edges per block so all cores run an identical (SPMD) program.
- GCN layer = xw matmul (weights folded with BN affine), per-edge gather of
  dinv-scaled rows via indirect DMA, segment-sum via selection-matrix matmuls
  accumulating in PSUM, then per-node affine + ReLU.
- Cross-core: AllGather of the dinv*(h@W) tables between layers; AllReduce of
  per-graph partial sums for mean-pooling. MLP head computed redundantly on all
  cores; core 0's output is returned.
"""

import sys

sys.path.insert(0, "/opt/trn_rl_repo")

import numpy as np

import concourse.bass as bass
import concourse.tile as tile
from concourse import bacc, mybir
from concourse import bass_utils
from concourse.bass import AP, IndirectOffsetOnAxis
from concourse.masks import make_identity

N_NODES = 50000
N_EDGES = 800000
NUM_GRAPHS = 256
D = 64
EPS = 1e-5
NCORES = 8
SH = N_NODES // NCORES          # 6250 nodes per core
BLK = 49                        # dst blocks of 128 per core
SHP = BLK * 128                 # 6272 padded shard size
NF = NCORES * SHP               # 50176 padded global size
P = 128

F32 = mybir.dt.float32
I32 = mybir.dt.int32
AF = mybir.ActivationFunctionType


def _host_prep(x, edge_index, batch):
    """Shard + preprocess index data on host. Returns per-core dicts + T."""
    x = np.asarray(x, dtype=np.float32)
    ei = np.asarray(edge_index, dtype=np.int64)
    batch = np.asarray(batch, dtype=np.int64)

    loops = np.arange(N_NODES, dtype=np.int64)
    src_all = np.concatenate([ei[0], loops])
    dst_all = np.concatenate([ei[1], loops])

    deg = np.bincount(dst_all, minlength=N_NODES).astype(np.float64)
    dinv = (1.0 / np.sqrt(np.maximum(deg, 1.0))).astype(np.float32)
    dinv[deg == 0] = 0.0

    # remap node id -> padded-concat layout (rank*SHP + local)
    r = src_all // SH
    src_rm = (r * SHP + (src_all - r * SH)).astype(np.int64)
    PADIDX = NF - 1  # a pad row (guaranteed zero in the u tables)

    cnt = np.bincount(batch, minlength=NUM_GRAPHS).astype(np.float32)
    pscale_g = (1.0 / np.maximum(cnt, 1.0)).astype(np.float32)

    # per-core edge bucketing
    per_core = []
    maxcnt = 0
    for c in range(NCORES):
        sel = (dst_all >= c * SH) & (dst_all < (c + 1) * SH)
        ls = (dst_all[sel] - c * SH).astype(np.int64)
        srcs = src_rm[sel]
        order = np.argsort(ls, kind="stable")
        ls = ls[order]
        srcs = srcs[order]
        blk = ls // P
        slot = (ls % P).astype(np.float32)
        cnts = np.bincount(blk, minlength=BLK)
        maxcnt = max(maxcnt, int(cnts.max()))
        per_core.append((srcs, blk, slot, cnts))
    T = (maxcnt + P - 1) // P

    maps = []
    for c in range(NCORES):
        srcs, blk, slot, cnts = per_core[c]
        srcpad = np.full((BLK, T * P), PADIDX, dtype=np.int32)
        slotpad = np.zeros((BLK, T * P), dtype=np.float32)
        starts = np.zeros(BLK + 1, dtype=np.int64)
        np.cumsum(cnts, out=starts[1:])
        for b in range(BLK):
            n = int(cnts[b])
            srcpad[b, :n] = srcs[starts[b]:starts[b] + n]
            slotpad[b, :n] = slot[starts[b]:starts[b] + n]
        # SBUF layout: [p, b*T + t] = entry (b, t*128 + p)
        srci = srcpad.reshape(BLK, T, P).transpose(2, 0, 1).reshape(P, BLK * T)
        slots = slotpad.reshape(BLK, T, P).transpose(2, 0, 1).reshape(P, BLK * T)

        dinv_sh = np.zeros(SHP, np.float32)
        dinv_sh[:SH] = dinv[c * SH:(c + 1) * SH]
        gid_sh = np.zeros(SHP, np.float32)
        gid_sh[:SH] = batch[c * SH:(c + 1) * SH].astype(np.float32)
        psc_sh = np.zeros(SHP, np.float32)
        psc_sh[:SH] = pscale_g[batch[c * SH:(c + 1) * SH]]

        xT_sh = np.zeros((D, SHP), np.float32)
        xT_sh[:, :SH] = x[c * SH:(c + 1) * SH].T

        maps.append({
            "xT": np.ascontiguousarray(xT_sh),
            "srci": np.ascontiguousarray(srci),
            "slot": np.ascontiguousarray(slots),
            "dinv": np.ascontiguousarray(dinv_sh.reshape(BLK, P).T),
            "gid": np.ascontiguousarray(gid_sh.reshape(BLK, P).T),
            "pscale": np.ascontiguousarray(psc_sh.reshape(BLK, P).T),
        })
    return maps, T


def _fold_weights(inp):
    """Fold BN affines into matmul weights / bias rows. Returns shared input map."""
    g = lambda k: np.asarray(inp[k], dtype=np.float32)
    A1 = g("bn_g0") / np.sqrt(g("bn_v0") + EPS)
    W0p = g("gcn_W0") * A1[None, :]
    C1 = A1 * (g("gcn_b0") - g("bn_m0")) + g("bn_b0")
    A2 = g("bn_g1") / np.sqrt(g("bn_v1") + EPS)
    W1p = g("gcn_W1") * A2[None, :]
    C2 = A2 * (g("gcn_b1") - g("bn_m1")) + g("bn_b1")

    A3 = g("hg1") / np.sqrt(g("hv1") + EPS)
    W1h = g("hW1") * A3[None, :]
    c1 = A3 * (g("hb1") - g("hm1")) + g("hbb1")
    A4 = g("hg2") / np.sqrt(g("hv2") + EPS)
    W2h = g("hW2") * A4[None, :]
    c2 = A4 * (g("hb2") - g("hm2")) + g("hbb2")

    iota128 = np.tile(np.arange(P, dtype=np.float32)[None, :], (P, 1))
    iota256 = np.tile(np.arange(2 * P, dtype=np.float32)[None, :], (P, 1))

    return {
        "w0p": W0p, "w1p": W1p,
        "c1bc": np.tile(C1[None, :], (P, 1)),
        "c2bc": np.tile(C2[None, :], (P, 1)),
        "iota128": iota128, "iota256": iota256,
        "w1h": W1h, "c1row": c1[None, :],
        "w2h": W2h, "c2row": c2[None, :],
        "w3h": g("hW3"), "b3row": g("hb3")[None, :],
        "w4h": g("hW4"), "b4bc": np.tile(g("hb4")[:, None], (P, 1)),
        "onesrow": np.ones((1, P), np.float32),
    }


def build_bass(T, debug=False):
    nc = bacc.Bacc(num_devices=NCORES)

    # I/O
    xT = nc.dram_tensor("xT", [D, SHP], F32, kind="ExternalInput")
    srci = nc.dram_tensor("srci", [P, BLK * T], I32, kind="ExternalInput")
    slot = nc.dram_tensor("slot", [P, BLK * T], F32, kind="ExternalInput")
    dinv = nc.dram_tensor("dinv", [P, BLK], F32, kind="ExternalInput")
    gid = nc.dram_tensor("gid", [P, BLK], F32, kind="ExternalInput")
    pscale = nc.dram_tensor("pscale", [P, BLK], F32, kind="ExternalInput")
    w0p = nc.dram_tensor("w0p", [D, D], F32, kind="ExternalInput")
    w1p = nc.dram_tensor("w1p", [D, D], F32, kind="ExternalInput")
    c1bc = nc.dram_tensor("c1bc", [P, D], F32, kind="ExternalInput")
    c2bc = nc.dram_tensor("c2bc", [P, D], F32, kind="ExternalInput")
    iota128 = nc.dram_tensor("iota128", [P, P], F32, kind="ExternalInput")
    iota256 = nc.dram_tensor("iota256", [P, 2 * P], F32, kind="ExternalInput")
    w1h = nc.dram_tensor("w1h", [D, 256], F32, kind="ExternalInput")
    c1row = nc.dram_tensor("c1row", [1, 256], F32, kind="ExternalInput")
    w2h = nc.dram_tensor("w2h", [256, P], F32, kind="ExternalInput")
    c2row = nc.dram_tensor("c2row", [1, P], F32, kind="ExternalInput")
    w3h = nc.dram_tensor("w3h", [P, D], F32, kind="ExternalInput")
    b3row = nc.dram_tensor("b3row", [1, D], F32, kind="ExternalInput")
    w4h = nc.dram_tensor("w4h", [D, 1], F32, kind="ExternalInput")
    b4bc = nc.dram_tensor("b4bc", [P, 1], F32, kind="ExternalInput")
    onesrow = nc.dram_tensor("onesrow", [1, P], F32, kind="ExternalInput")
    out = nc.dram_tensor("out", [2 * P, 1], F32, kind="ExternalOutput")

    # internal DRAM
    u1sh = nc.dram_tensor("u1sh", [SHP, D], F32)
    u1f = nc.dram_tensor("u1f", [NF, D], F32, addr_space="Shared")
    u2sh = nc.dram_tensor("u2sh", [SHP, D], F32)
    u2f = nc.dram_tensor("u2f", [NF, D], F32, addr_space="Shared")
    poolin = nc.dram_tensor("poolin", [P, P], F32)
    poolout = nc.dram_tensor("poolout", [P, P], F32, addr_space="Shared")
    if debug:
        u1fo = nc.dram_tensor("u1fo", [NF, D], F32, kind="ExternalOutput")
        u2fo = nc.dram_tensor("u2fo", [NF, D], F32, kind="ExternalOutput")
        poolo = nc.dram_tensor("poolo", [P, P], F32, kind="ExternalOutput")

    rg = [list(range(NCORES))]

    def bcast3(ap_2d, t_count, inner, inner_step):
        """[P, t_count] AP -> [P, t_count, inner] with given inner step (0=broadcast)."""
        return AP(ap_2d.tensor, ap_2d.offset,
                  [[ap_2d.ap[0][0], P], [ap_2d.ap[1][0], t_count], [inner_step, inner]])

    with tile.TileContext(nc) as tc:
        with tc.tile_pool(name="const", bufs=1) as cp, \
             tc.tile_pool(name="work", bufs=4) as wp, \
             tc.tile_pool(name="msg", bufs=3) as mp, \
             tc.tile_pool(name="sel", bufs=2) as sp, \
             tc.tile_pool(name="psA", bufs=3, space="PSUM") as psA, \
             tc.tile_pool(name="psB", bufs=2, space="PSUM") as psB, \
             tc.tile_pool(name="psPool", bufs=1, space="PSUM") as psP:

            # --- constants resident in SBUF ---
            w0p_sb = cp.tile([D, D], F32); nc.sync.dma_start(out=w0p_sb[:], in_=w0p[:])
            w1p_sb = cp.tile([D, D], F32); nc.sync.dma_start(out=w1p_sb[:], in_=w1p[:])
            dinv_sb = cp.tile([P, BLK], F32); nc.sync.dma_start(out=dinv_sb[:], in_=dinv[:])
            srci_sb = cp.tile([P, BLK * T], I32); nc.sync.dma_start(out=srci_sb[:], in_=srci[:])
            slot_sb = cp.tile([P, BLK * T], F32); nc.sync.dma_start(out=slot_sb[:], in_=slot[:])
            iota128_sb = cp.tile([P, P], F32); nc.sync.dma_start(out=iota128_sb[:], in_=iota128[:])
            iota256_sb = cp.tile([P, 2 * P], F32); nc.sync.dma_start(out=iota256_sb[:], in_=iota256[:])
            c1bc_sb = cp.tile([P, D], F32); nc.sync.dma_start(out=c1bc_sb[:], in_=c1bc[:])
            c2bc_sb = cp.tile([P, D], F32); nc.sync.dma_start(out=c2bc_sb[:], in_=c2bc[:])
            gid_sb = cp.tile([P, BLK], F32); nc.sync.dma_start(out=gid_sb[:], in_=gid[:])
            pscale_sb = cp.tile([P, BLK], F32); nc.sync.dma_start(out=pscale_sb[:], in_=pscale[:])
            ident = cp.tile([P, P], F32); make_identity(nc, ident[:])

            # --- phase A: u1 = dinv * (x @ W0') per shard ---
            for b in range(BLK):
                xt = wp.tile([D, P], F32, tag="xt")
                nc.sync.dma_start(out=xt[:], in_=xT[:, b * P:(b + 1) * P])
                pu = psA.tile([P, D], F32, space="PSUM", tag="acc")
                nc.tensor.matmul(out=pu[:], lhsT=xt[:], rhs=w0p_sb[:], start=True, stop=True)
                us = wp.tile([P, D], F32, tag="us")
                nc.vector.tensor_scalar_mul(us[:], pu[:], dinv_sb[:, b:b + 1])
                nc.sync.dma_start(out=u1sh[b * P:(b + 1) * P, :], in_=us[:])

            nc.gpsimd.collective_compute(
                "AllGather", mybir.AluOpType.bypass, replica_groups=rg,
                ins=[u1sh[:, :].opt()], outs=[u1f[:, :].opt()])
            if debug:
                nc.sync.dma_start(out=u1fo[:, :], in_=u1f[:, :])

            def seg_layer(ufull, b, cbc_sb):
                """gather + segment-sum one dst block; returns relu'd h tile [128, 64]."""
                msg = mp.tile([P, T * D], F32, tag="msg")
                for t in range(T):
                    j = b * T + t
                    nc.gpsimd.indirect_dma_start(
                        out=msg[:, t * D:(t + 1) * D], out_offset=None,
                        in_=ufull[:, :],
                        in_offset=IndirectOffsetOnAxis(ap=srci_sb[:, j:j + 1], axis=0))
                S = sp.tile([P, T * P], F32, tag="S")
                S_ap = S[:]
                sl = slot_sb[:, b * T:(b + 1) * T]
                nc.vector.tensor_tensor(
                    out=AP(S_ap.tensor, S_ap.offset, [[S_ap.ap[0][0], P], [P, T], [1, P]]),
                    in0=bcast3(sl, T, P, 0),
                    in1=AP(iota128_sb[:].tensor, iota128_sb[:].offset,
                           [[iota128_sb[:].ap[0][0], P], [0, T], [1, P]]),
                    op=mybir.AluOpType.is_equal)
                hpb = psA.tile([P, D], F32, space="PSUM", tag="acc")
                for t in range(T):
                    nc.tensor.matmul(out=hpb[:], lhsT=S[:, t * P:(t + 1) * P],
                                     rhs=msg[:, t * D:(t + 1) * D],
                                     start=(t == 0), stop=(t == T - 1))
                t1 = wp.tile([P, D], F32, tag="t1")
                nc.vector.tensor_scalar_mul(t1[:], hpb[:], dinv_sb[:, b:b + 1])
                t2 = wp.tile([P, D], F32, tag="t2")
                nc.vector.tensor_add(t2[:], t1[:], cbc_sb[:])
                h = wp.tile([P, D], F32, tag="h")
                nc.scalar.activation(h[:], t2[:], AF.Relu)
                return h

            # --- phase C: layer-1 message passing + fused u2 compute ---
            for b in range(BLK):
                h1 = seg_layer(u1f, b, c1bc_sb)
                tpb = psB.tile([D, P], F32, space="PSUM", tag="tr")
                nc.tensor.transpose(out=tpb[:], in_=h1[:], identity=ident[:])
                h1T = wp.tile([D, P], F32, tag="h1T")
                nc.vector.tensor_copy(out=h1T[:], in_=tpb[:])
                upb = psB.tile([P, D], F32, space="PSUM", tag="tr")
                nc.tensor.matmul(out=upb[:], lhsT=h1T[:], rhs=w1p_sb[:], start=True, stop=True)
                u2s = wp.tile([P, D], F32, tag="u2s")
                nc.vector.tensor_scalar_mul(u2s[:], upb[:], dinv_sb[:, b:b + 1])
                nc.sync.dma_start(out=u2sh[b * P:(b + 1) * P, :], in_=u2s[:])

            nc.gpsimd.collective_compute(
                "AllGather", mybir.AluOpType.bypass, replica_groups=rg,
                ins=[u2sh[:, :].opt()], outs=[u2f[:, :].opt()])
            if debug:
                nc.sync.dma_start(out=u2fo[:, :], in_=u2f[:, :])

            # --- phase E: layer-2 message passing + pooling ---
            pp0 = psP.tile([P, D], F32, space="PSUM", tag="pp0")
            pp1 = psP.tile([P, D], F32, space="PSUM", tag="pp1")
            for b in range(BLK):
                h2 = seg_layer(u2f, b, c2bc_sb)
                Sg = wp.tile([P, 2 * P], F32, tag="Sg")
                nc.vector.tensor_tensor(
                    out=Sg[:], in0=gid_sb[:, b:b + 1].to_broadcast([P, 2 * P]),
                    in1=iota256_sb[:], op=mybir.AluOpType.is_equal)
                Sgs = wp.tile([P, 2 * P], F32, tag="Sgs")
                nc.vector.tensor_scalar_mul(Sgs[:], Sg[:], pscale_sb[:, b:b + 1])
                nc.tensor.matmul(out=pp0[:], lhsT=Sgs[:, 0:P], rhs=h2[:],
                                 start=(b == 0), stop=(b == BLK - 1), skip_group_check=True)
                nc.tensor.matmul(out=pp1[:], lhsT=Sgs[:, P:2 * P], rhs=h2[:],
                                 start=(b == 0), stop=(b == BLK - 1), skip_group_check=True)

            pool_sb = wp.tile([P, P], F32, tag="pool")
            nc.vector.tensor_copy(out=pool_sb[:, 0:D], in_=pp0[:])
            nc.vector.tensor_copy(out=pool_sb[:, D:P], in_=pp1[:])
            nc.sync.dma_start(out=poolin[:, :], in_=pool_sb[:])
            nc.gpsimd.collective_compute(
                "AllReduce", mybir.AluOpType.add, replica_groups=rg,
                ins=[poolin[:, :].opt()], outs=[poolout[:, :].opt()])
            if debug:
                nc.sync.dma_start(out=poolo[:, :], in_=poolout[:, :])

            # --- phase G: MLP head (redundant on all cores) ---
            w1h_sb = cp.tile([D, 256], F32); nc.sync.dma_start(out=w1h_sb[:], in_=w1h[:])
            c1row_sb = cp.tile([1, 256], F32); nc.sync.dma_start(out=c1row_sb[:], in_=c1row[:])
            w2h_sb0 = cp.tile([P, P], F32); nc.sync.dma_start(out=w2h_sb0[:], in_=w2h[0:P, :])
            w2h_sb1 = cp.tile([P, P], F32); nc.sync.dma_start(out=w2h_sb1[:], in_=w2h[P:2 * P, :])
            c2row_sb = cp.tile([1, P], F32); nc.sync.dma_start(out=c2row_sb[:], in_=c2row[:])
            w3h_sb = cp.tile([P, D], F32); nc.sync.dma_start(out=w3h_sb[:], in_=w3h[:])
            b3row_sb = cp.tile([1, D], F32); nc.sync.dma_start(out=b3row_sb[:], in_=b3row[:])
            w4h_sb = cp.tile([D, 1], F32); nc.sync.dma_start(out=w4h_sb[:], in_=w4h[:])
            b4bc_sb = cp.tile([P, 1], F32); nc.sync.dma_start(out=b4bc_sb[:], in_=b4bc[:])
            ones_sb = cp.tile([1, P], F32); nc.sync.dma_start(out=ones_sb[:], in_=onesrow[:])

            pooled = wp.tile([P, P], F32, tag="pooled")
            nc.sync.dma_start(out=pooled[:], in_=poolout[:, :])
            pT = []
            for m in range(2):
                tp = psB.tile([D, P], F32, space="PSUM", tag="tr")
                nc.tensor.transpose(out=tp[:], in_=pooled[:, m * D:(m + 1) * D], identity=ident[:])
                pTm = wp.tile([D, P], F32, tag=f"pT{m}")
                nc.vector.tensor_copy(out=pTm[:], in_=tp[:])
                pT.append(pTm)

            z1 = []
            for m in range(2):
                z1p = psA.tile([P, 256], F32, space="PSUM", tag="acc")
                nc.tensor.matmul(out=z1p[:], lhsT=pT[m][:], rhs=w1h_sb[:], start=True, stop=False)
                nc.tensor.matmul(out=z1p[:], lhsT=ones_sb[:], rhs=c1row_sb[:], start=False, stop=True)
                z1m = wp.tile([P, 256], F32, tag=f"z1{m}")
                nc.scalar.activation(z1m[:], z1p[:], AF.Relu)
                z1.append(z1m)
            z1T = []
            for k in range(2):
                z1Tk = wp.tile([P, 256], F32, tag=f"z1T{k}")
                for m in range(2):
                    tp = psB.tile([P, P], F32, space="PSUM", tag="tr")
                    nc.tensor.transpose(out=tp[:], in_=z1[m][:, k * P:(k + 1) * P], identity=ident[:])
                    nc.vector.tensor_copy(out=z1Tk[:, m * P:(m + 1) * P], in_=tp[:])
                z1T.append(z1Tk)

            z2 = []
            for m in range(2):
                z2p = psA.tile([P, P], F32, space="PSUM", tag="acc")
                nc.tensor.matmul(out=z2p[:], lhsT=z1T[0][:, m * P:(m + 1) * P], rhs=w2h_sb0[:], start=True, stop=False)
                nc.tensor.matmul(out=z2p[:], lhsT=z1T[1][:, m * P:(m + 1) * P], rhs=w2h_sb1[:], start=False, stop=False)
                nc.tensor.matmul(out=z2p[:], lhsT=ones_sb[:], rhs=c2row_sb[:], start=False, stop=True)
                z2m = wp.tile([P, P], F32, tag=f"z2{m}")
                nc.scalar.activation(z2m[:], z2p[:], AF.Relu)
                z2.append(z2m)
            z2T = wp.tile([P, 2 * P], F32, tag="z2T")
            for m in range(2):
                tp = psB.tile([P, P], F32, space="PSUM", tag="tr")
                nc.tensor.transpose(out=tp[:], in_=z2[m][:], identity=ident[:])
                nc.vector.tensor_copy(out=z2T[:, m * P:(m + 1) * P], in_=tp[:])

            for m in range(2):
                z3p = psA.tile([P, D], F32, space="PSUM", tag="acc")
                nc.tensor.matmul(out=z3p[:], lhsT=z2T[:, m * P:(m + 1) * P], rhs=w3h_sb[:], start=True, stop=False)
                nc.tensor.matmul(out=z3p[:], lhsT=ones_sb[:], rhs=b3row_sb[:], start=False, stop=True)
                z3m = wp.tile([P, D], F32, tag="z3m")
                nc.scalar.activation(z3m[:], z3p[:], AF.Relu)
                tp = psB.tile([D, P], F32, space="PSUM", tag="tr")
                nc.tensor.transpose(out=tp[:], in_=z3m[:], identity=ident[:])
                z3T = wp.tile([D, P], F32, tag="z3T")
                nc.vector.tensor_copy(out=z3T[:], in_=tp[:])
                z4p = psA.tile([P, 1], F32, space="PSUM", tag="acc")
                nc.tensor.matmul(out=z4p[:], lhsT=z3T[:], rhs=w4h_sb[:], start=True, stop=True)
                z4 = wp.tile([P, 1], F32, tag="z4")
                nc.vector.tensor_scalar_add(z4[:], z4p[:], b4bc_sb[:, 0:1])
                nc.sync.dma_start(out=out[m * P:(m + 1) * P, :], in_=z4[:])

    nc.compile()
    return nc


_CACHE = {}


def kernel(**inputs):
    maps, T = _host_prep(inputs["x"], inputs["edge_index"], inputs["batch"])
    shared = _fold_weights(inputs)

    key = T
    if key not in _CACHE:
        _CACHE[key] = build_bass(T)
    nc = _CACHE[key]

    in_maps = [{**m, **shared} for m in maps]
    res = bass_utils.run_bass_kernel_spmd(nc, in_maps, core_ids=list(range(NCORES)))
    return res.results[0]["out"].reshape(-1).astype(np.float32)



